# revision 29
# baseline (speedup 1.0000x reference)
"""Location-sensitive attention on 8 Trainium2 NeuronCores.

Reference computation (B=32, T=2048, D=1024, conv kernel K=31):
    loc = conv1d(stack([prev_attn, cum_attn]), Wloc, pad=15)   # [B,T,D]
    q   = query @ Wq.T                                          # [B,1,D]
    m   = memory @ Wm.T                                         # [B,T,D]
    e   = tanh(q + m + loc) @ v                                 # [B,T]
    e   = where(mask, -inf, e); a = softmax(e, axis=1)
    ctx = einsum('bt,btd->bd', a, memory)
Returns (ctx, a).

Sharding: batch B data-parallel across 8 cores (4 batches/core), weights
replicated. Device-side layout is "feature-major": memory is fed
pre-transposed as memT[b] = memory[b].T (shape [D,T], bf16) so the big
matmul contracts d on partitions, and the location conv is a 62-wide
matmul against an im2col matrix built on the host.

Per (e_chunk of 128, t_block of 512) the kernel accumulates in PSUM:
    psum = sum_k WmT[k,e].T @ memT[k,t]  +  WlocT[:,e].T @ X[:,t]
evicts via ScalarE tanh with per-partition bias q2[e] (= Wq@query), and
reduces over e with a [128,1] v matmul accumulated across e_chunks into
a [1,512] PSUM row (mask bias folded in as a K=1 matmul). Softmax
normalization is deferred: exp(e) is broadcast to 128 partitions via a
K=1 PE matmul and ctx_unnorm[d] = sum_t exp(e_t) memT[d,t] accumulates
on VectorE per t-block; both outputs are scaled by 1/Z at the end, so
only a few microseconds of work trail the last matmul.
"""

import numpy as np
import ml_dtypes

B, T, D, KW = 32, 2048, 1024, 31
NCORES = 8
BPC = B // NCORES        # batches per core
PAD = KW // 2
F = 2 * KW               # im2col features
P = 128
KC = D // P              # contraction chunks
EC = D // P              # output-feature chunks
NT = 512                 # matmul free-dim tile
TB = T // NT             # t blocks
MASK_NEG = -50.0         # exp(-50+e) ~ 0; reference uses -inf

_cache = {}


def _build_nc():
    import concourse.bacc as bacc
    import concourse.mybir as mybir
    import concourse.tile as tile
    from concourse.bass import ts

    bf16 = mybir.dt.bfloat16
    f32 = mybir.dt.float32
    AF = mybir.ActivationFunctionType
    AX = mybir.AxisListType

    nc = bacc.Bacc("TRN2", target_bir_lowering=False, debug=False)
    memT = nc.declare_dram_parameter("memT", [BPC, D, T], bf16, isOutput=False)
    X = nc.declare_dram_parameter("X", [BPC, F, T], bf16, isOutput=False)
    queryT = nc.declare_dram_parameter("queryT", [D, BPC], bf16, isOutput=False)
    WmT = nc.declare_dram_parameter("WmT", [D, D], bf16, isOutput=False)
    WqT = nc.declare_dram_parameter("WqT", [D, D], bf16, isOutput=False)
    WlocT = nc.declare_dram_parameter("WlocT", [F, D], bf16, isOutput=False)
    vW = nc.declare_dram_parameter("vW", [D], bf16, isOutput=False)
    maskb = nc.declare_dram_parameter("maskb", [1, BPC, T], bf16, isOutput=False)
    ctx_out = nc.declare_dram_parameter("ctx_out", [BPC, D], f32, isOutput=True)
    a_out = nc.declare_dram_parameter("a_out", [BPC, T], f32, isOutput=True)

    with tile.TileContext(nc) as tc:
        from contextlib import ExitStack

        with ExitStack() as st:
            wpool = st.enter_context(tc.tile_pool(name="wpool", bufs=1))
            mempool = st.enter_context(tc.tile_pool(name="mempool", bufs=3))
            xpool = st.enter_context(tc.tile_pool(name="xpool", bufs=2))
            hpool = st.enter_context(tc.tile_pool(name="hpool", bufs=4))
            small = st.enter_context(tc.tile_pool(name="small", bufs=2))
            ebcpool = st.enter_context(tc.tile_pool(name="ebcpool", bufs=4))
            scratch = st.enter_context(tc.tile_pool(name="scratch", bufs=2))
            mpsum = st.enter_context(tc.tile_pool(name="mpsum", bufs=3, space="PSUM"))
            bpsum = st.enter_context(tc.tile_pool(name="bpsum", bufs=1, space="PSUM"))
            epsum = st.enter_context(tc.tile_pool(name="epsum", bufs=2, space="PSUM"))

            # ---- one-time loads; qT/wq first so q2 matmuls start early ----
            qT = wpool.tile([P, KC, BPC], bf16, name="qT")
            nc.sync.dma_start(qT, queryT[:].rearrange("(k p) b -> p k b", p=P))
            wq = wpool.tile([P, KC, D], bf16, name="wq")
            wq_r = WqT[:].rearrange("(k p) e -> p k e", p=P)
            for k in range(KC):
                nc.sync.dma_start(wq[:, k, :], wq_r[:, k, :])

            # q2[e, b] = (Wq @ query_b)[e] — also warms up the PE clock
            q2 = wpool.tile([P, EC, BPC], f32, name="q2")
            for e in range(EC):
                psq = mpsum.tile([P, NT], f32, name="psq", tag="mps")
                for k in range(KC):
                    nc.tensor.matmul(
                        psq[:, :BPC], wq[:, k, ts(e, P)], qT[:, k, :],
                        start=(k == 0), stop=(k == KC - 1),
                    )
                nc.scalar.copy(q2[:, e, :], psq[:, :BPC])

            wm = wpool.tile([P, KC, D], bf16, name="wm")
            wm_r = WmT[:].rearrange("(k p) e -> p k e", p=P)
            for k in range(KC):
                nc.sync.dma_start(wm[:, k, :], wm_r[:, k, :])
            wloc = wpool.tile([F, D], bf16, name="wloc")
            nc.sync.dma_start(wloc, WlocT[:])
            vt = wpool.tile([P, EC], bf16, name="vt")
            nc.sync.dma_start(vt, vW[:].rearrange("(c p) -> p c", p=P))
            ones = wpool.tile([1, P], bf16, name="ones")
            nc.vector.memset(ones, 1.0)
            ones_f = wpool.tile([1, P], f32, name="ones_f")
            nc.vector.memset(ones_f, 1.0)

            # ---- per-batch pipeline ---------------------------------------
            # Deferred-op software pipeline: PE is in-order, so a vdot (or
            # epilogue matmul) placed right after its producing group would
            # stall PE on ScalarE's tanh/exp. Instead each dependent op is
            # queued and emitted one mp-group later (~1.9us of cover).
            from collections import deque

            deferred = deque()

            def flush_one():
                if deferred:
                    deferred.popleft()()

            for b in range(BPC):
                mt = mempool.tile([P, KC, T], bf16, name="mt", tag="mt")
                mem_r = memT[b].rearrange("(k p) t -> p k t", p=P)
                xt = xpool.tile([F, T], bf16, name="xt", tag="xt")
                nc.sync.dma_start(xt, X[b])
                mbt = small.tile([1, T], bf16, name="mbt", tag="mbt")
                nc.sync.dma_start(mbt, maskb[:, b, :])
                for k in range(KC):
                    nc.sync.dma_start(mt[:, k, :], mem_r[:, k, :])

                expt = small.tile([1, T], f32, name="expt", tag="expt")
                parts = small.tile([1, TB], f32, name="parts", tag="parts")
                ctxp = small.tile([P, KC, TB], f32, name="ctxp", tag="ctxp")

                for tb in range(TB):
                    eps = epsum.tile([1, NT], f32, name="eps", tag="eps")
                    for e in range(EC):
                        mp = mpsum.tile([P, NT], f32, name="mp", tag="mps")
                        for k in range(KC):
                            nc.tensor.matmul(
                                mp, wm[:, k, ts(e, P)], mt[:, k, ts(tb, NT)],
                                start=(k == 0), stop=False,
                            )
                        nc.tensor.matmul(
                            mp, wloc[:, ts(e, P)], xt[:, ts(tb, NT)],
                            start=False, stop=True,
                        )
                        ht = hpool.tile([P, NT], bf16, name="ht", tag="ht")
                        nc.scalar.activation(
                            ht, mp, AF.Tanh, bias=q2[:, e, b : b + 1]
                        )
                        flush_one()
                        if e < EC - 1:
                            deferred.append(
                                lambda eps=eps, e=e, ht=ht: nc.tensor.matmul(
                                    eps, vt[:, e : e + 1], ht,
                                    start=(e == 0), stop=False,
                                )
                            )

                    hold = {}

                    def close_pack(eps=eps, ht=ht, tb=tb, b=b, mbt=mbt,
                                   expt=expt, parts=parts, hold=hold):
                        # last vdot, mask add (closes group), exp + row-sum
                        nc.tensor.matmul(
                            eps, vt[:, EC - 1 : EC], ht,
                            start=False, stop=False,
                        )
                        nc.tensor.matmul(
                            eps, ones[:, 0:1], mbt[:, ts(tb, NT)],
                            start=False, stop=True,
                        )
                        nc.scalar.activation(
                            expt[:, ts(tb, NT)], eps, AF.Exp,
                            accum_out=parts[:, tb : tb + 1],
                        )
                        ebf = small.tile([1, NT], bf16, name="ebf", tag="ebf")
                        nc.scalar.copy(ebf, expt[:, ts(tb, NT)])
                        hold["ebf"] = ebf

                    def bc_pack(hold=hold, tb=tb, mt=mt, ctxp=ctxp):
                        # broadcast exp row to 128 partitions, ctx partials
                        bp = bpsum.tile([P, NT], f32, name="bp", tag="bp")
                        nc.tensor.matmul(bp, ones, hold["ebf"], start=True, stop=True)
                        ebc = ebcpool.tile([P, NT], bf16, name="ebc", tag="ebc")
                        nc.scalar.copy(ebc, bp)
                        for k in range(KC):
                            junk = scratch.tile(
                                [P, NT], bf16, name="junk", tag="junk"
                            )
                            nc.vector.tensor_mul(
                                junk, mt[:, k, ts(tb, NT)], ebc
                            )
                            nc.vector.reduce_sum(
                                ctxp[:, k, tb : tb + 1], junk, axis=AX.X
                            )

                    deferred.append(close_pack)
                    deferred.append(bc_pack)

                def norm_pack(b=b, expt=expt, parts=parts, ctxp=ctxp):
                    # Z, 1/Z, scale a and ctx, write outputs
                    zsum = small.tile([1, 1], f32, name="zsum", tag="zsum")
                    nc.vector.reduce_sum(zsum, parts, axis=AX.X)
                    rz = small.tile([1, 1], f32, name="rz", tag="rz")
                    nc.vector.reciprocal(rz, zsum)
                    arow = small.tile([1, T], f32, name="arow", tag="arow")
                    nc.vector.tensor_scalar_mul(arow, expt, rz)
                    nc.sync.dma_start(a_out[b : b + 1, :], arow)
                    rzb = bpsum.tile([P, 1], f32, name="rzb", tag="bp")
                    nc.tensor.matmul(rzb, ones_f, rz, start=True, stop=True)
                    rzbc = small.tile([P, 1], f32, name="rzbc", tag="rzbc")
                    nc.scalar.copy(rzbc, rzb)
                    ctxr = small.tile([P, KC], f32, name="ctxr", tag="ctxr")
                    nc.vector.reduce_sum(ctxr, ctxp, axis=AX.X)
                    ctxt = small.tile([P, KC], f32, name="ctxt", tag="ctxt")
                    nc.vector.tensor_scalar_mul(ctxt, ctxr, rzbc)
                    nc.sync.dma_start(
                        ctx_out[b].rearrange("(k p) -> p k", p=P), ctxt
                    )

                deferred.append(norm_pack)

            while deferred:
                flush_one()
    nc.compile()
    return nc


def _get_nc():
    if "nc" not in _cache:
        _cache["nc"] = _build_nc()
    return _cache["nc"]


def _prep_inputs(query, memory, prev_attn, cum_attn, mask):
    bf = ml_dtypes.bfloat16
    memTb = np.ascontiguousarray(memory.transpose(0, 2, 1)).astype(bf)  # [B,D,T]
    loc_in = np.stack([prev_attn, cum_attn], axis=1).astype(np.float32)  # [B,2,T]
    padded = np.pad(loc_in, ((0, 0), (0, 0), (PAD, PAD)))
    X = np.stack(
        [padded[:, c, k : k + T] for c in range(2) for k in range(KW)], axis=1
    ).astype(bf)  # [B, 62, T]
    maskb = np.where(mask, np.float32(MASK_NEG), np.float32(0.0)).astype(bf)
    return memTb, X, maskb


def kernel(query, memory, prev_attn, cum_attn, mask, Wq, Wm, Wloc, v, _trace=False):
    from concourse.bass_utils import run_bass_kernel_spmd

    bf = ml_dtypes.bfloat16
    nc = _get_nc()
    memTb, X, maskb = _prep_inputs(query, memory, prev_attn, cum_attn, mask)
    WmT_ = np.ascontiguousarray(np.asarray(Wm).T).astype(bf)
    WqT_ = np.ascontiguousarray(np.asarray(Wq).T).astype(bf)
    WlocT_ = np.ascontiguousarray(
        np.asarray(Wloc).transpose(1, 2, 0).reshape(F, D)
    ).astype(bf)
    v_ = np.asarray(v).astype(bf)

    in_maps = []
    for c in range(NCORES):
        s = slice(c * BPC, (c + 1) * BPC)
        qTc = np.ascontiguousarray(np.asarray(query)[s].T).astype(bf)  # [D, BPC]
        in_maps.append(
            {
                "memT": memTb[s],
                "X": X[s],
                "queryT": qTc,
                "WmT": WmT_,
                "WqT": WqT_,
                "WlocT": WlocT_,
                "vW": v_,
                "maskb": maskb[s][None],
            }
        )
    kwargs = {}
    if _trace:
        kwargs = {"trace": True, "trace_cores": [0]}
    res = run_bass_kernel_spmd(nc, in_maps, core_ids=list(range(NCORES)), **kwargs)
    _cache["last_results"] = res
    ctx = np.concatenate([r["ctx_out"] for r in res.results], axis=0)
    a = np.concatenate([r["a_out"] for r in res.results], axis=0)
    return ctx.astype(np.float32), a.astype(np.float32)


# revision 33
# speedup vs baseline: 1.0815x; 1.0815x over previous
"""Location-sensitive attention on 8 Trainium2 NeuronCores.

Reference computation (B=32, T=2048, D=1024, conv kernel K=31):
    loc = conv1d(stack([prev_attn, cum_attn]), Wloc, pad=15)   # [B,T,D]
    q   = query @ Wq.T                                          # [B,1,D]
    m   = memory @ Wm.T                                         # [B,T,D]
    e   = tanh(q + m + loc) @ v                                 # [B,T]
    e   = where(mask, -inf, e); a = softmax(e, axis=1)
    ctx = einsum('bt,btd->bd', a, memory)
Returns (ctx, a).

Sharding: batch B data-parallel across 8 cores (4 batches/core), weights
replicated. Device-side layout is "feature-major": memory is fed
pre-transposed as memT[b] = memory[b].T (shape [D,T], bf16) so the big
matmul contracts d on partitions, and the location conv is a 62-wide
matmul against an im2col matrix built on the host.

Per (e_chunk of 128, t_block of 512) the kernel accumulates in PSUM:
    psum = sum_k WmT[k,e].T @ memT[k,t]  +  WlocT[:,e].T @ X[:,t]
evicts via ScalarE tanh with per-partition bias q2[e] (= Wq@query), and
reduces over e with a [128,1] v matmul accumulated across e_chunks into
a [1,512] PSUM row (mask bias folded in as a K=1 matmul). Softmax
normalization is deferred: exp(e) is broadcast to 128 partitions via a
K=1 PE matmul and ctx_unnorm[d] = sum_t exp(e_t) memT[d,t] accumulates
on VectorE per t-block; both outputs are scaled by 1/Z at the end, so
only a few microseconds of work trail the last matmul.
"""

import numpy as np
import ml_dtypes

B, T, D, KW = 32, 2048, 1024, 31
NCORES = 8
BPC = B // NCORES        # batches per core
PAD = KW // 2
F = 2 * KW               # im2col features
P = 128
KC = D // P              # contraction chunks
EC = D // P              # output-feature chunks
NT = 512                 # matmul free-dim tile
TB = T // NT             # t blocks
MASK_NEG = -50.0         # exp(-50+e) ~ 0; reference uses -inf

_cache = {}


def _build_nc():
    import concourse.bacc as bacc
    import concourse.mybir as mybir
    import concourse.tile as tile
    from concourse.bass import ts

    bf16 = mybir.dt.bfloat16
    f32 = mybir.dt.float32
    AF = mybir.ActivationFunctionType
    AX = mybir.AxisListType

    nc = bacc.Bacc("TRN2", target_bir_lowering=False, debug=False)
    memT = nc.declare_dram_parameter("memT", [BPC, D, T], bf16, isOutput=False)
    X = nc.declare_dram_parameter("X", [BPC, F, T], bf16, isOutput=False)
    queryT = nc.declare_dram_parameter("queryT", [D, BPC], bf16, isOutput=False)
    WmT = nc.declare_dram_parameter("WmT", [D, D], bf16, isOutput=False)
    WqT = nc.declare_dram_parameter("WqT", [D, D], bf16, isOutput=False)
    WlocT = nc.declare_dram_parameter("WlocT", [F, D], bf16, isOutput=False)
    vW = nc.declare_dram_parameter("vW", [D], bf16, isOutput=False)
    maskb = nc.declare_dram_parameter("maskb", [1, BPC, T], bf16, isOutput=False)
    ctx_out = nc.declare_dram_parameter("ctx_out", [BPC, D], f32, isOutput=True)
    a_out = nc.declare_dram_parameter("a_out", [BPC, T], f32, isOutput=True)

    with tile.TileContext(nc) as tc:
        from contextlib import ExitStack

        with ExitStack() as st:
            wpool = st.enter_context(tc.tile_pool(name="wpool", bufs=1))
            mempool = st.enter_context(tc.tile_pool(name="mempool", bufs=3))
            xpool = st.enter_context(tc.tile_pool(name="xpool", bufs=2))
            hpool = st.enter_context(tc.tile_pool(name="hpool", bufs=4))
            small = st.enter_context(tc.tile_pool(name="small", bufs=2))
            ebcpool = st.enter_context(tc.tile_pool(name="ebcpool", bufs=4))
            scratch = st.enter_context(tc.tile_pool(name="scratch", bufs=2))
            mpsum = st.enter_context(tc.tile_pool(name="mpsum", bufs=3, space="PSUM"))
            bpsum = st.enter_context(tc.tile_pool(name="bpsum", bufs=1, space="PSUM"))
            epsum = st.enter_context(tc.tile_pool(name="epsum", bufs=2, space="PSUM"))

            # ---- one-time loads; qT/wq first so q2 matmuls start early ----
            qT = wpool.tile([P, KC, BPC], bf16, name="qT")
            nc.sync.dma_start(qT, queryT[:].rearrange("(k p) b -> p k b", p=P))
            wq = wpool.tile([P, KC, D], bf16, name="wq")
            wq_r = WqT[:].rearrange("(k p) e -> p k e", p=P)
            for k in range(KC):
                nc.sync.dma_start(wq[:, k, :], wq_r[:, k, :])

            # q2[e, b] = (Wq @ query_b)[e] — also warms up the PE clock
            q2 = wpool.tile([P, EC, BPC], f32, name="q2")
            for e in range(EC):
                psq = mpsum.tile([P, NT], f32, name="psq", tag="mps")
                for k in range(KC):
                    nc.tensor.matmul(
                        psq[:, :BPC], wq[:, k, ts(e, P)], qT[:, k, :],
                        start=(k == 0), stop=(k == KC - 1),
                    )
                nc.scalar.copy(q2[:, e, :], psq[:, :BPC])

            wm = wpool.tile([P, KC, D], bf16, name="wm")
            wm_r = WmT[:].rearrange("(k p) e -> p k e", p=P)
            for k in range(KC):
                nc.sync.dma_start(wm[:, k, :], wm_r[:, k, :])
            wloc = wpool.tile([F, D], bf16, name="wloc")
            nc.sync.dma_start(wloc, WlocT[:])
            vt = wpool.tile([P, EC], bf16, name="vt")
            nc.sync.dma_start(vt, vW[:].rearrange("(c p) -> p c", p=P))
            vt_f = wpool.tile([P, EC], f32, name="vt_f")
            nc.vector.tensor_copy(vt_f, vt)
            ones = wpool.tile([1, P], bf16, name="ones")
            nc.vector.memset(ones, 1.0)
            ones_f = wpool.tile([1, P], f32, name="ones_f")
            nc.vector.memset(ones_f, 1.0)
            ones_col = wpool.tile([P, 1], bf16, name="ones_col")
            nc.vector.memset(ones_col, 1.0)

            # ---- per-batch pipeline ---------------------------------------
            # Deferred-op software pipeline: PE is in-order, so a vdot (or
            # epilogue matmul) placed right after its producing group would
            # stall PE on ScalarE's tanh/exp. Instead each dependent op is
            # queued and emitted one mp-group later (~1.9us of cover).
            from collections import deque

            deferred = deque()

            def flush_one():
                if deferred:
                    deferred.popleft()()

            for b in range(BPC):
                mt = mempool.tile([P, KC, T], bf16, name="mt", tag="mt")
                mem_r = memT[b].rearrange("(k p) t -> p k t", p=P)
                xt = xpool.tile([F, T], bf16, name="xt", tag="xt")
                nc.sync.dma_start(xt, X[b])
                mbt = small.tile([1, T], bf16, name="mbt", tag="mbt")
                nc.sync.dma_start(mbt, maskb[:, b, :])
                for k in range(KC):
                    nc.sync.dma_start(mt[:, k, :], mem_r[:, k, :])

                expt = small.tile([1, T], f32, name="expt", tag="expt")
                parts = small.tile([1, TB], f32, name="parts", tag="parts")
                ctxp = small.tile([P, KC, TB], f32, name="ctxp", tag="ctxp")

                for tb in range(TB):
                    # acc[p, t] accumulates v[e]*tanh(...) per partition on
                    # VectorE; the cross-partition sum becomes ONE matmul per
                    # tb instead of 8 — PE sheds ~28 matmuls per batch.
                    acc = ebcpool.tile([P, NT], f32, name="acc", tag="acc",
                                       bufs=2)
                    for e in range(EC):
                        mp = mpsum.tile([P, NT], f32, name="mp", tag="mps")
                        for k in range(KC):
                            nc.tensor.matmul(
                                mp, wm[:, k, ts(e, P)], mt[:, k, ts(tb, NT)],
                                start=(k == 0), stop=False,
                            )
                        nc.tensor.matmul(
                            mp, wloc[:, ts(e, P)], xt[:, ts(tb, NT)],
                            start=False, stop=True,
                        )
                        ht = hpool.tile([P, NT], bf16, name="ht", tag="ht")
                        nc.scalar.activation(
                            ht, mp, AF.Tanh, bias=q2[:, e, b : b + 1]
                        )
                        flush_one()
                        if e == 0:
                            nc.vector.tensor_scalar_mul(acc, ht, vt_f[:, 0:1])
                        else:
                            hv = scratch.tile([P, NT], f32, name="hv",
                                              tag="hv", bufs=1)
                            nc.vector.tensor_scalar_mul(
                                hv, ht, vt_f[:, e : e + 1]
                            )
                            nc.vector.tensor_add(acc, acc, hv)
                    acc_bf = ebcpool.tile([P, NT], bf16, name="acc_bf",
                                          tag="acc_bf", bufs=2)
                    nc.scalar.copy(acc_bf, acc)

                    hold = {}

                    def close_pack(acc_bf=acc_bf, tb=tb, b=b, mbt=mbt,
                                   expt=expt, parts=parts, hold=hold):
                        # cross-partition v-sum, mask add, exp + row-sum
                        eps = epsum.tile([1, NT], f32, name="eps", tag="eps")
                        nc.tensor.matmul(
                            eps, ones_col, acc_bf,
                            start=True, stop=False,
                        )
                        nc.tensor.matmul(
                            eps, ones[:, 0:1], mbt[:, ts(tb, NT)],
                            start=False, stop=True,
                        )
                        nc.scalar.activation(
                            expt[:, ts(tb, NT)], eps, AF.Exp,
                            accum_out=parts[:, tb : tb + 1],
                        )
                        ebf = small.tile([1, NT], bf16, name="ebf", tag="ebf")
                        nc.scalar.copy(ebf, expt[:, ts(tb, NT)])
                        hold["ebf"] = ebf

                    def bc_pack(hold=hold, tb=tb, mt=mt, ctxp=ctxp):
                        # broadcast exp row to 128 partitions, ctx partials
                        bp = bpsum.tile([P, NT], f32, name="bp", tag="bp")
                        nc.tensor.matmul(bp, ones, hold["ebf"], start=True, stop=True)
                        ebc = ebcpool.tile([P, NT], bf16, name="ebc", tag="ebc")
                        nc.scalar.copy(ebc, bp)
                        for k in range(KC):
                            junk = scratch.tile(
                                [P, NT], bf16, name="junk", tag="junk"
                            )
                            nc.vector.tensor_mul(
                                junk, mt[:, k, ts(tb, NT)], ebc
                            )
                            nc.vector.reduce_sum(
                                ctxp[:, k, tb : tb + 1], junk, axis=AX.X
                            )

                    deferred.append(close_pack)
                    deferred.append(bc_pack)

                def norm_pack(b=b, expt=expt, parts=parts, ctxp=ctxp):
                    # Z, 1/Z, scale a and ctx, write outputs
                    zsum = small.tile([1, 1], f32, name="zsum", tag="zsum")
                    nc.vector.reduce_sum(zsum, parts, axis=AX.X)
                    rz = small.tile([1, 1], f32, name="rz", tag="rz")
                    nc.vector.reciprocal(rz, zsum)
                    arow = small.tile([1, T], f32, name="arow", tag="arow")
                    nc.vector.tensor_scalar_mul(arow, expt, rz)
                    nc.sync.dma_start(a_out[b : b + 1, :], arow)
                    rzb = bpsum.tile([P, 1], f32, name="rzb", tag="bp")
                    nc.tensor.matmul(rzb, ones_f, rz, start=True, stop=True)
                    rzbc = small.tile([P, 1], f32, name="rzbc", tag="rzbc")
                    nc.scalar.copy(rzbc, rzb)
                    ctxr = small.tile([P, KC], f32, name="ctxr", tag="ctxr")
                    nc.vector.reduce_sum(ctxr, ctxp, axis=AX.X)
                    ctxt = small.tile([P, KC], f32, name="ctxt", tag="ctxt")
                    nc.vector.tensor_scalar_mul(ctxt, ctxr, rzbc)
                    nc.sync.dma_start(
                        ctx_out[b].rearrange("(k p) -> p k", p=P), ctxt
                    )

                deferred.append(norm_pack)

            while deferred:
                flush_one()
    nc.compile()
    return nc


def _get_nc():
    if "nc" not in _cache:
        _cache["nc"] = _build_nc()
    return _cache["nc"]


def _prep_inputs(query, memory, prev_attn, cum_attn, mask):
    bf = ml_dtypes.bfloat16
    memTb = np.ascontiguousarray(memory.transpose(0, 2, 1)).astype(bf)  # [B,D,T]
    loc_in = np.stack([prev_attn, cum_attn], axis=1).astype(np.float32)  # [B,2,T]
    padded = np.pad(loc_in, ((0, 0), (0, 0), (PAD, PAD)))
    X = np.stack(
        [padded[:, c, k : k + T] for c in range(2) for k in range(KW)], axis=1
    ).astype(bf)  # [B, 62, T]
    maskb = np.where(mask, np.float32(MASK_NEG), np.float32(0.0)).astype(bf)
    return memTb, X, maskb


def kernel(query, memory, prev_attn, cum_attn, mask, Wq, Wm, Wloc, v, _trace=False):
    from concourse.bass_utils import run_bass_kernel_spmd

    bf = ml_dtypes.bfloat16
    nc = _get_nc()
    memTb, X, maskb = _prep_inputs(query, memory, prev_attn, cum_attn, mask)
    WmT_ = np.ascontiguousarray(np.asarray(Wm).T).astype(bf)
    WqT_ = np.ascontiguousarray(np.asarray(Wq).T).astype(bf)
    WlocT_ = np.ascontiguousarray(
        np.asarray(Wloc).transpose(1, 2, 0).reshape(F, D)
    ).astype(bf)
    v_ = np.asarray(v).astype(bf)

    in_maps = []
    for c in range(NCORES):
        s = slice(c * BPC, (c + 1) * BPC)
        qTc = np.ascontiguousarray(np.asarray(query)[s].T).astype(bf)  # [D, BPC]
        in_maps.append(
            {
                "memT": memTb[s],
                "X": X[s],
                "queryT": qTc,
                "WmT": WmT_,
                "WqT": WqT_,
                "WlocT": WlocT_,
                "vW": v_,
                "maskb": maskb[s][None],
            }
        )
    kwargs = {}
    if _trace:
        kwargs = {"trace": True, "trace_cores": [0]}
    res = run_bass_kernel_spmd(nc, in_maps, core_ids=list(range(NCORES)), **kwargs)
    _cache["last_results"] = res
    ctx = np.concatenate([r["ctx_out"] for r in res.results], axis=0)
    a = np.concatenate([r["a_out"] for r in res.results], axis=0)
    return ctx.astype(np.float32), a.astype(np.float32)


# revision 34
# speedup vs baseline: 1.0864x; 1.0045x over previous
"""Location-sensitive attention on 8 Trainium2 NeuronCores.

Reference computation (B=32, T=2048, D=1024, conv kernel K=31):
    loc = conv1d(stack([prev_attn, cum_attn]), Wloc, pad=15)   # [B,T,D]
    q   = query @ Wq.T                                          # [B,1,D]
    m   = memory @ Wm.T                                         # [B,T,D]
    e   = tanh(q + m + loc) @ v                                 # [B,T]
    e   = where(mask, -inf, e); a = softmax(e, axis=1)
    ctx = einsum('bt,btd->bd', a, memory)
Returns (ctx, a).

Sharding: batch B data-parallel across 8 cores (4 batches/core), weights
replicated. Device-side layout is "feature-major": memory is fed
pre-transposed as memT[b] = memory[b].T (shape [D,T], bf16) so the big
matmul contracts d on partitions, and the location conv is a 62-wide
matmul against an im2col matrix built on the host.

Per (e_chunk of 128, t_block of 512) the kernel accumulates in PSUM:
    psum = sum_k WmT[k,e].T @ memT[k,t]  +  WlocT[:,e].T @ X[:,t]
evicts via ScalarE tanh with per-partition bias q2[e] (= Wq@query), and
reduces over e with a [128,1] v matmul accumulated across e_chunks into
a [1,512] PSUM row (mask bias folded in as a K=1 matmul). Softmax
normalization is deferred: exp(e) is broadcast to 128 partitions via a
K=1 PE matmul and ctx_unnorm[d] = sum_t exp(e_t) memT[d,t] accumulates
on VectorE per t-block; both outputs are scaled by 1/Z at the end, so
only a few microseconds of work trail the last matmul.
"""

import numpy as np
import ml_dtypes

B, T, D, KW = 32, 2048, 1024, 31
NCORES = 8
BPC = B // NCORES        # batches per core
PAD = KW // 2
F = 2 * KW               # im2col features
P = 128
KC = D // P              # contraction chunks
EC = D // P              # output-feature chunks
NT = 512                 # matmul free-dim tile
TB = T // NT             # t blocks
MASK_NEG = -50.0         # exp(-50+e) ~ 0; reference uses -inf

_cache = {}


def _build_nc():
    import concourse.bacc as bacc
    import concourse.mybir as mybir
    import concourse.tile as tile
    from concourse.bass import ts

    bf16 = mybir.dt.bfloat16
    f32 = mybir.dt.float32
    AF = mybir.ActivationFunctionType
    AX = mybir.AxisListType

    nc = bacc.Bacc("TRN2", target_bir_lowering=False, debug=False)
    memT = nc.declare_dram_parameter("memT", [BPC, D, T], bf16, isOutput=False)
    X = nc.declare_dram_parameter("X", [BPC, F, T], bf16, isOutput=False)
    queryT = nc.declare_dram_parameter("queryT", [D, BPC], bf16, isOutput=False)
    WmT = nc.declare_dram_parameter("WmT", [D, D], bf16, isOutput=False)
    WqT = nc.declare_dram_parameter("WqT", [D, D], bf16, isOutput=False)
    WlocT = nc.declare_dram_parameter("WlocT", [F, D], bf16, isOutput=False)
    vW = nc.declare_dram_parameter("vW", [D], bf16, isOutput=False)
    maskb = nc.declare_dram_parameter("maskb", [1, BPC, T], bf16, isOutput=False)
    ctx_out = nc.declare_dram_parameter("ctx_out", [BPC, D], f32, isOutput=True)
    a_out = nc.declare_dram_parameter("a_out", [BPC, T], f32, isOutput=True)

    with tile.TileContext(nc) as tc:
        from contextlib import ExitStack

        with ExitStack() as st:
            wpool = st.enter_context(tc.tile_pool(name="wpool", bufs=1))
            mempool = st.enter_context(tc.tile_pool(name="mempool", bufs=3))
            xpool = st.enter_context(tc.tile_pool(name="xpool", bufs=2))
            hpool = st.enter_context(tc.tile_pool(name="hpool", bufs=4))
            small = st.enter_context(tc.tile_pool(name="small", bufs=2))
            ebcpool = st.enter_context(tc.tile_pool(name="ebcpool", bufs=4))
            scratch = st.enter_context(tc.tile_pool(name="scratch", bufs=2))
            mpsum = st.enter_context(tc.tile_pool(name="mpsum", bufs=3, space="PSUM"))
            bpsum = st.enter_context(tc.tile_pool(name="bpsum", bufs=1, space="PSUM"))
            epsum = st.enter_context(tc.tile_pool(name="epsum", bufs=2, space="PSUM"))

            # ---- one-time loads; qT/wq first so q2 matmuls start early ----
            qT = wpool.tile([P, KC, BPC], bf16, name="qT")
            nc.sync.dma_start(qT, queryT[:].rearrange("(k p) b -> p k b", p=P))
            wq = wpool.tile([P, KC, D], bf16, name="wq")
            wq_r = WqT[:].rearrange("(k p) e -> p k e", p=P)
            for k in range(KC):
                nc.sync.dma_start(wq[:, k, :], wq_r[:, k, :])

            # q2[e, b] = (Wq @ query_b)[e] — also warms up the PE clock
            q2 = wpool.tile([P, EC, BPC], f32, name="q2")
            for e in range(EC):
                psq = mpsum.tile([P, NT], f32, name="psq", tag="mps")
                for k in range(KC):
                    nc.tensor.matmul(
                        psq[:, :BPC], wq[:, k, ts(e, P)], qT[:, k, :],
                        start=(k == 0), stop=(k == KC - 1),
                    )
                nc.scalar.copy(q2[:, e, :], psq[:, :BPC])

            wm = wpool.tile([P, KC, D], bf16, name="wm")
            wm_r = WmT[:].rearrange("(k p) e -> p k e", p=P)
            for k in range(KC):
                nc.sync.dma_start(wm[:, k, :], wm_r[:, k, :])
            wloc = wpool.tile([F, D], bf16, name="wloc")
            nc.sync.dma_start(wloc, WlocT[:])
            vt = wpool.tile([P, EC], bf16, name="vt")
            nc.sync.dma_start(vt, vW[:].rearrange("(c p) -> p c", p=P))
            vt_f = wpool.tile([P, EC], f32, name="vt_f")
            nc.vector.tensor_copy(vt_f, vt)
            ones = wpool.tile([1, P], bf16, name="ones")
            nc.vector.memset(ones, 1.0)
            ones_f = wpool.tile([1, P], f32, name="ones_f")
            nc.vector.memset(ones_f, 1.0)
            ones_col = wpool.tile([P, 1], bf16, name="ones_col")
            nc.vector.memset(ones_col, 1.0)

            # ---- per-batch pipeline ---------------------------------------
            # Deferred-op software pipeline: PE is in-order, so a vdot (or
            # epilogue matmul) placed right after its producing group would
            # stall PE on ScalarE's tanh/exp. Instead each dependent op is
            # queued and emitted one mp-group later (~1.9us of cover).
            from collections import deque

            deferred = deque()

            def flush_one():
                if deferred:
                    deferred.popleft()()

            for b in range(BPC):
                mt = mempool.tile([P, KC, T], bf16, name="mt", tag="mt")
                mem_r = memT[b].rearrange("(k p) t -> p k t", p=P)
                xt = xpool.tile([F, T], bf16, name="xt", tag="xt")
                nc.sync.dma_start(xt, X[b])
                mbt = small.tile([1, T], bf16, name="mbt", tag="mbt")
                nc.sync.dma_start(mbt, maskb[:, b, :])
                for k in range(KC):
                    nc.sync.dma_start(mt[:, k, :], mem_r[:, k, :])

                expt = small.tile([1, T], f32, name="expt", tag="expt")
                parts = small.tile([1, TB], f32, name="parts", tag="parts")
                ctxp = small.tile([P, KC, TB], f32, name="ctxp", tag="ctxp")

                for tb in range(TB):
                    # acc[p, t] accumulates v[e]*tanh(...) per partition on
                    # VectorE; the cross-partition sum becomes ONE matmul per
                    # tb instead of 8 — PE sheds ~28 matmuls per batch.
                    acc = ebcpool.tile([P, NT], f32, name="acc", tag="acc",
                                       bufs=2)
                    for e in range(EC):
                        mp = mpsum.tile([P, NT], f32, name="mp", tag="mps")
                        for k in range(KC):
                            nc.tensor.matmul(
                                mp, wm[:, k, ts(e, P)], mt[:, k, ts(tb, NT)],
                                start=(k == 0), stop=False,
                            )
                        nc.tensor.matmul(
                            mp, wloc[:, ts(e, P)], xt[:, ts(tb, NT)],
                            start=False, stop=True,
                        )
                        ht = hpool.tile([P, NT], bf16, name="ht", tag="ht")
                        nc.scalar.activation(
                            ht, mp, AF.Tanh, bias=q2[:, e, b : b + 1]
                        )
                        flush_one()
                        if e == 0:
                            nc.vector.tensor_scalar_mul(acc, ht, vt_f[:, 0:1])
                        else:
                            hv = scratch.tile([P, NT], bf16, name="hv",
                                              tag="hv", bufs=1)
                            nc.vector.tensor_scalar_mul(
                                hv, ht, vt_f[:, e : e + 1]
                            )
                            nc.vector.tensor_add(acc, acc, hv)
                    acc_bf = ebcpool.tile([P, NT], bf16, name="acc_bf",
                                          tag="acc_bf", bufs=2)
                    nc.scalar.copy(acc_bf, acc)

                    hold = {}

                    def close_pack(acc_bf=acc_bf, tb=tb, b=b, mbt=mbt,
                                   expt=expt, parts=parts, hold=hold):
                        # cross-partition v-sum, mask add, exp + row-sum
                        eps = epsum.tile([1, NT], f32, name="eps", tag="eps")
                        nc.tensor.matmul(
                            eps, ones_col, acc_bf,
                            start=True, stop=False,
                        )
                        nc.tensor.matmul(
                            eps, ones[:, 0:1], mbt[:, ts(tb, NT)],
                            start=False, stop=True,
                        )
                        nc.scalar.activation(
                            expt[:, ts(tb, NT)], eps, AF.Exp,
                            accum_out=parts[:, tb : tb + 1],
                        )
                        ebf = small.tile([1, NT], bf16, name="ebf", tag="ebf")
                        nc.scalar.copy(ebf, expt[:, ts(tb, NT)])
                        hold["ebf"] = ebf

                    def bc_pack(hold=hold, tb=tb, mt=mt, ctxp=ctxp):
                        # broadcast exp row to 128 partitions, ctx partials
                        bp = bpsum.tile([P, NT], f32, name="bp", tag="bp")
                        nc.tensor.matmul(bp, ones, hold["ebf"], start=True, stop=True)
                        ebc = ebcpool.tile([P, NT], bf16, name="ebc", tag="ebc")
                        nc.scalar.copy(ebc, bp)
                        for k in range(KC):
                            junk = scratch.tile(
                                [P, NT], bf16, name="junk", tag="junk"
                            )
                            nc.vector.tensor_mul(
                                junk, mt[:, k, ts(tb, NT)], ebc
                            )
                            nc.vector.reduce_sum(
                                ctxp[:, k, tb : tb + 1], junk, axis=AX.X
                            )

                    deferred.append(close_pack)
                    deferred.append(bc_pack)

                def norm_pack(b=b, expt=expt, parts=parts, ctxp=ctxp):
                    # Z, 1/Z, scale a and ctx, write outputs
                    zsum = small.tile([1, 1], f32, name="zsum", tag="zsum")
                    nc.vector.reduce_sum(zsum, parts, axis=AX.X)
                    rz = small.tile([1, 1], f32, name="rz", tag="rz")
                    nc.vector.reciprocal(rz, zsum)
                    arow = small.tile([1, T], f32, name="arow", tag="arow")
                    nc.vector.tensor_scalar_mul(arow, expt, rz)
                    nc.sync.dma_start(a_out[b : b + 1, :], arow)
                    rzb = bpsum.tile([P, 1], f32, name="rzb", tag="bp")
                    nc.tensor.matmul(rzb, ones_f, rz, start=True, stop=True)
                    rzbc = small.tile([P, 1], f32, name="rzbc", tag="rzbc")
                    nc.scalar.copy(rzbc, rzb)
                    ctxr = small.tile([P, KC], f32, name="ctxr", tag="ctxr")
                    nc.vector.reduce_sum(ctxr, ctxp, axis=AX.X)
                    ctxt = small.tile([P, KC], f32, name="ctxt", tag="ctxt")
                    nc.vector.tensor_scalar_mul(ctxt, ctxr, rzbc)
                    nc.sync.dma_start(
                        ctx_out[b].rearrange("(k p) -> p k", p=P), ctxt
                    )

                deferred.append(norm_pack)

            while deferred:
                flush_one()
    nc.compile()
    return nc


def _get_nc():
    if "nc" not in _cache:
        _cache["nc"] = _build_nc()
    return _cache["nc"]


def _prep_inputs(query, memory, prev_attn, cum_attn, mask):
    bf = ml_dtypes.bfloat16
    memTb = np.ascontiguousarray(memory.transpose(0, 2, 1)).astype(bf)  # [B,D,T]
    loc_in = np.stack([prev_attn, cum_attn], axis=1).astype(np.float32)  # [B,2,T]
    padded = np.pad(loc_in, ((0, 0), (0, 0), (PAD, PAD)))
    X = np.stack(
        [padded[:, c, k : k + T] for c in range(2) for k in range(KW)], axis=1
    ).astype(bf)  # [B, 62, T]
    maskb = np.where(mask, np.float32(MASK_NEG), np.float32(0.0)).astype(bf)
    return memTb, X, maskb


def kernel(query, memory, prev_attn, cum_attn, mask, Wq, Wm, Wloc, v, _trace=False):
    from concourse.bass_utils import run_bass_kernel_spmd

    bf = ml_dtypes.bfloat16
    nc = _get_nc()
    memTb, X, maskb = _prep_inputs(query, memory, prev_attn, cum_attn, mask)
    WmT_ = np.ascontiguousarray(np.asarray(Wm).T).astype(bf)
    WqT_ = np.ascontiguousarray(np.asarray(Wq).T).astype(bf)
    WlocT_ = np.ascontiguousarray(
        np.asarray(Wloc).transpose(1, 2, 0).reshape(F, D)
    ).astype(bf)
    v_ = np.asarray(v).astype(bf)

    in_maps = []
    for c in range(NCORES):
        s = slice(c * BPC, (c + 1) * BPC)
        qTc = np.ascontiguousarray(np.asarray(query)[s].T).astype(bf)  # [D, BPC]
        in_maps.append(
            {
                "memT": memTb[s],
                "X": X[s],
                "queryT": qTc,
                "WmT": WmT_,
                "WqT": WqT_,
                "WlocT": WlocT_,
                "vW": v_,
                "maskb": maskb[s][None],
            }
        )
    kwargs = {}
    if _trace:
        kwargs = {"trace": True, "trace_cores": [0]}
    res = run_bass_kernel_spmd(nc, in_maps, core_ids=list(range(NCORES)), **kwargs)
    _cache["last_results"] = res
    ctx = np.concatenate([r["ctx_out"] for r in res.results], axis=0)
    a = np.concatenate([r["a_out"] for r in res.results], axis=0)
    return ctx.astype(np.float32), a.astype(np.float32)
